# revision 27
# baseline (speedup 1.0000x reference)
"""Trainium2 Bass kernel for nn_Attention_22187801051652.

Module: qkv = x @ w_qkv -> (B,N,3,H,D); per-token softmax over the HEAD axis
(H x H Gram of the 16 heads per token); o @ w_proj + b_proj.

Strategy: data-parallel over batch B=8 across the 8 NeuronCores (one batch
element per core, no collectives). Per core, all dense matmuls run in bf16
on the TensorEngine; the per-token 16x16 head-softmax is computed with
packed 8-token matmuls (128 = 8 tokens x 16 heads on the partition axis),
a block-diagonal mask applied after exp, and the softmax denominator taken
from a ones-column appended to the V operand of the O matmul. Attention
ACT/DVE ops are batched 4 groups at a time to amortize per-op overhead.

Self-contained: hardcodes B=8, N=4096, C=1024, H=16, D=64.
"""
from contextlib import ExitStack

import numpy as np
import ml_dtypes

import concourse.bass as bass
import concourse.bacc as bacc
import concourse.tile as tile
from concourse import mybir
from concourse.bass_utils import run_bass_kernel_spmd

F32 = mybir.dt.float32
BF16 = mybir.dt.bfloat16
AFT = mybir.ActivationFunctionType
bf16 = ml_dtypes.bfloat16

B, N, C, H, D = 8, 4096, 1024, 16, 64
SCALE = D ** -0.5
T = N          # tokens per core
ST = 256       # supertile


def _bcast(ap, count):
    """Append a step-0 (broadcast) innermost free dim to an AP."""
    return bass.AP(tensor=ap.tensor, offset=ap.offset, ap=list(ap.ap) + [[0, count]])


def _build(nc, T=T, ST=ST):
    out_t = nc.dram_tensor("out", [T, C], F32, kind="ExternalOutput")
    xt_t = nc.dram_tensor("xt", [C, T], BF16, kind="ExternalInput")
    wqkv_t = nc.dram_tensor("wqkv", [C, 3 * C], BF16, kind="ExternalInput")
    wproj_t = nc.dram_tensor("wproj", [8, 128, C], BF16, kind="ExternalInput")
    mask_t = nc.dram_tensor("mask", [128, 512], BF16, kind="ExternalInput")
    identb_t = nc.dram_tensor("identb", [128, 128], BF16, kind="ExternalInput")
    
    out_ap = out_t.ap()
    xt_ap = xt_t.ap()
    n_st = T // ST
    n_tt = ST // 128
    n_grp = ST // 8

    with tile.TileContext(nc) as tc, ExitStack() as ctx:
        const = ctx.enter_context(tc.tile_pool(name="const", bufs=1))
        xtp = ctx.enter_context(tc.tile_pool(name="xtp", bufs=2))
        stg = ctx.enter_context(tc.tile_pool(name="stg", bufs=4))
        flats = ctx.enter_context(tc.tile_pool(name="flats", bufs=2))
        attp = ctx.enter_context(tc.tile_pool(name="attp", bufs=3))
        otp = ctx.enter_context(tc.tile_pool(name="otp", bufs=2))
        vp1 = ctx.enter_context(tc.tile_pool(name="vp1", bufs=1))
        ps_att = ctx.enter_context(tc.tile_pool(name="ps_att", bufs=3, space="PSUM"))
        ps_pack = ctx.enter_context(tc.tile_pool(name="ps_pack", bufs=2, space="PSUM"))
        ps_big = ctx.enter_context(tc.tile_pool(name="ps_big", bufs=2, space="PSUM"))
        ps_vp = ctx.enter_context(tc.tile_pool(name="ps_vp", bufs=1, space="PSUM"))

        wq_sb = const.tile([128, 8, 3072], BF16)
        for kk in range(8):
            nc.sync.dma_start(out=wq_sb[:, kk, :],
                              in_=wqkv_t.ap()[kk * 128:(kk + 1) * 128, :])
        wp_sb = const.tile([128, 8, 1024], BF16)
        nc.sync.dma_start(out=wp_sb[:], in_=wproj_t.ap().rearrange("hp c j -> c hp j"))
        mask_sb = const.tile([128, 4, 128], BF16)
        nc.sync.dma_start(out=mask_sb[:], in_=mask_t.ap().rearrange("p (i c) -> p i c", i=4))
        idb = const.tile([128, 128], BF16)
        nc.sync.dma_start(out=idb[:], in_=identb_t.ap())

        for st in range(n_st):
            t0 = st * ST
            # ---- X^T: direct load of host-transposed bf16 x ----
            xt_sb = xtp.tile([128, 8, ST], BF16, tag="xt")
            for kk in range(8):
                nc.sync.dma_start(
                    out=xt_sb[:, kk, :],
                    in_=xt_ap[kk * 128:(kk + 1) * 128, t0:t0 + ST])

            # ---- qkv matmuls -> staging -> repack to flats ----
            qf = flats.tile([64, 16, ST], BF16, tag="qf")
            kf = flats.tile([64, 16, ST], BF16, tag="kf")
            # vstage: group-contiguous [p=(g%2)*64+d, tgrp, (gp*8+t)]
            vstage = vp1.tile([128, ST // 8, 64], BF16, tag="vs")
            for jj in range(24):
                qkv_ps = ps_big.tile([128, ST], F32, tag="big")
                for kk in range(8):
                    nc.tensor.matmul(
                        qkv_ps[:], wq_sb[:, kk, jj * 128:(jj + 1) * 128],
                        xt_sb[:, kk, :], start=(kk == 0), stop=(kk == 7))
                if jj < 16:
                    dst = qf if jj < 8 else kf
                    hh = 2 * (jj % 8)
                    sg = stg.tile([128, ST], BF16, tag="stg")
                    nc.vector.tensor_copy(sg[:], qkv_ps[:])
                    eng = nc.sync if jj < 8 else nc.scalar
                    eng.dma_start(out=dst[:, hh, :], in_=sg[0:64, :])
                    eng.dma_start(out=dst[:, hh + 1, :], in_=sg[64:128, :])
                else:
                    gp = jj - 16
                    nc.vector.tensor_copy(
                        vstage[:, :, gp * 8:(gp + 1) * 8],
                        qkv_ps[:].rearrange("p (tg t) -> p tg t", t=8))

            # ---- attention groups, batched 4 at a time ----
            kap4_all = kf[:].rearrange(
                "d (gp g1) (gg i t) -> d gg i g1 gp t", g1=2, i=4, t=8)
            qap_all = qf[:].rearrange("d (hp h1) t -> d h1 hp t", h1=2)
            otflats = []
            otdsts = []
            for tt in range(n_tt):
                otf_tile = otp.tile([128, 8, 128], BF16, tag=f"otf{tt}")
                otflats.append(otf_tile)
                otdsts.append(otf_tile[:].rearrange("p hp (gg i t) -> p gg i hp t", i=4, t=8))

            # V-pack: dense runs of PE transposes per half-supertile,
            # one evict + ones column each.
            vpk = vp1.tile([128, n_grp, 65], BF16, tag="vpk")
            hg = n_grp // 4
            for vh in range(4):
                vp_ps = ps_vp.tile([128, hg, 64], BF16, tag="vp")
                for g in range(hg):
                    grp = vh * hg + g
                    nc.tensor.transpose(vp_ps[0:64, g, :], vstage[0:64, grp, :], idb[0:64, 0:64])
                    nc.tensor.transpose(vp_ps[64:128, g, :], vstage[64:128, grp, :], idb[64:128, 64:128])
                nc.scalar.copy(vpk[:, vh * hg:(vh + 1) * hg, 0:64], vp_ps[:])
            nc.vector.memset(vpk[:, :, 64:65], 1.0)

            for g4 in range(n_grp // 4):
                # K-pack for 4 groups in one copy (stationary needs 1 free dim)
                kp4 = attp.tile([64, 4, 128], BF16, tag="kp")
                nc.vector.tensor_copy(
                    kp4[:].rearrange("d i (g1 gp t) -> d i g1 gp t", g1=2, gp=8),
                    kap4_all[:, g4])

                st_ps = ps_att.tile([128, 4, 128], F32, tag="att")
                for i in range(4):
                    tsl = bass.ts(g4 * 4 + i, 8)
                    nc.tensor.matmul(
                        st_ps[:, i, :], kp4[:, i, :], qap_all[:, :, :, tsl],
                        start=True, stop=True)
                e_sb = attp.tile([128, 4, 128], BF16, tag="e")
                nc.scalar.activation(e_sb[:], st_ps[:], AFT.Exp)
                p_bdt = attp.tile([128, 4, 128], BF16, tag="p")
                nc.vector.tensor_mul(p_bdt[:], e_sb[:], mask_sb[:])

                o_ps = ps_att.tile([128, 4, 65], F32, tag="att")
                for i in range(4):
                    nc.tensor.matmul(o_ps[:, i, :], p_bdt[:, i, :],
                                     vpk[:, g4 * 4 + i, :],
                                     start=True, stop=True)
                recip = attp.tile([128, 4, 1], F32, tag="recip")
                nc.vector.reciprocal(recip[:], o_ps[:, :, 64:65])
                o_norm = attp.tile([128, 4, 64], BF16, tag="onorm")
                nc.vector.tensor_mul(
                    o_norm[:], o_ps[:, :, 0:64],
                    _bcast(recip[:, :, 0], 64))

                ot_ps = ps_pack.tile([128, 4, 64], BF16, tag="pack")
                for i in range(4):
                    nc.tensor.transpose(ot_ps[0:64, i, :], o_norm[0:64, i, :], idb[0:64, 0:64])
                    nc.tensor.transpose(ot_ps[64:128, i, :], o_norm[64:128, i, :], idb[64:128, 64:128])
                tt_i, g4_i = divmod(g4, 4)
                nc.scalar.copy(otdsts[tt_i][0:64, g4_i], ot_ps[0:64, :, :])
                nc.scalar.copy(otdsts[tt_i][64:128, g4_i], ot_ps[64:128, :, :])

            # ---- proj ----
            for tt in range(n_tt):
                for nh in range(2):
                    prj_ps = ps_big.tile([128, 512], F32, tag="big")
                    for hp in range(8):
                        nc.tensor.matmul(
                            prj_ps[:], otflats[tt][:, hp, :],
                            wp_sb[:, hp, nh * 512:(nh + 1) * 512],
                            start=(hp == 0), stop=(hp == 7))
                    o_out = stg.tile([128, 512], F32, tag="oout")
                    nc.any.tensor_copy(o_out[:], prj_ps[:])
                    nc.sync.dma_start(
                        out=out_ap[t0 + tt * 128: t0 + (tt + 1) * 128,
                                   nh * 512:(nh + 1) * 512],
                        in_=o_out[:])
    return nc


_CACHE = {}


def _get_nc():
    if "nc" not in _CACHE:
        nc = bacc.Bacc("TRN2", target_bir_lowering=False, debug=False,
                       enable_asserts=False, num_devices=8)
        _build(nc)
        nc.compile()
        _CACHE["nc"] = nc
    return _CACHE["nc"]


def _host_prep(w_qkv, w_proj):
    wqkv_mod = np.asarray(w_qkv, np.float32).copy()
    wqkv_mod[:, :C] *= SCALE
    wqkv_b = wqkv_mod.astype(bf16)
    wproj_b = np.ascontiguousarray(
        np.asarray(w_proj, np.float32).reshape(8, 128, C)).astype(bf16)
    mask = np.zeros((128, 128), np.float32)
    for g in range(H):
        for t in range(8):
            pg = (g % 2) * 64 + (g // 2) * 8 + t
            for h in range(H):
                ph = (h % 2) * 64 + (h // 2) * 8 + t
                mask[pg, ph] = 1.0
    mask4 = np.tile(mask, (1, 4)).astype(bf16)
    identb = np.eye(128, dtype=np.float32).astype(bf16)
    identf = np.eye(128, dtype=np.float32)
    return wqkv_b, wproj_b, mask4, identb, identf


def _run(x, w_qkv, w_proj, b_proj, trace=False, **kw):
    x = np.asarray(x, np.float32)
    wqkv_b, wproj_b, mask4, identb, identf = _host_prep(w_qkv, w_proj)
    nc = _get_nc()
    in_maps = []
    for b in range(B):
        in_maps.append({
            "xt": np.ascontiguousarray(x[b].T).astype(bf16),
            "wqkv": wqkv_b, "wproj": wproj_b, "mask": mask4,
            "identb": identb,
        })
    res = run_bass_kernel_spmd(nc, in_maps, core_ids=list(range(B)),
                               trace=trace, **kw)
    out = np.stack([res.results[b]["out"] for b in range(B)], axis=0)
    out = out + np.asarray(b_proj, np.float32)[None, None, :]
    return out.astype(np.float32), res


def kernel(x, w_qkv, w_proj, b_proj):
    out, _ = _run(x, w_qkv, w_proj, b_proj, trace=False)
    return out


# revision 29
# speedup vs baseline: 1.0246x; 1.0246x over previous
"""Trainium2 Bass kernel for nn_Attention_22187801051652.

Module: qkv = x @ w_qkv -> (B,N,3,H,D); per-token softmax over the HEAD axis
(H x H Gram of the 16 heads per token); o @ w_proj + b_proj.

Strategy: data-parallel over batch B=8 across the 8 NeuronCores (one batch
element per core, no collectives). Per core, all dense matmuls run in bf16
on the TensorEngine; the per-token 16x16 head-softmax is computed with
packed 8-token matmuls (128 = 8 tokens x 16 heads on the partition axis),
a block-diagonal mask applied after exp, and the softmax denominator taken
from a ones-column appended to the V operand of the O matmul. Attention
ACT/DVE ops are batched 4 groups at a time to amortize per-op overhead.

Self-contained: hardcodes B=8, N=4096, C=1024, H=16, D=64.
"""
from contextlib import ExitStack

import numpy as np
import ml_dtypes

import concourse.bass as bass
import concourse.bacc as bacc
import concourse.tile as tile
from concourse import mybir
from concourse.bass_utils import run_bass_kernel_spmd

F32 = mybir.dt.float32
BF16 = mybir.dt.bfloat16
AFT = mybir.ActivationFunctionType
bf16 = ml_dtypes.bfloat16

B, N, C, H, D = 8, 4096, 1024, 16, 64
SCALE = D ** -0.5
T = N          # tokens per core
ST = 256       # supertile


def _bcast(ap, count):
    """Append a step-0 (broadcast) innermost free dim to an AP."""
    return bass.AP(tensor=ap.tensor, offset=ap.offset, ap=list(ap.ap) + [[0, count]])


def _build(nc, T=T, ST=ST):
    out_t = nc.dram_tensor("out", [T, C], F32, kind="ExternalOutput")
    xt_t = nc.dram_tensor("xt", [C, T], BF16, kind="ExternalInput")
    wqkv_t = nc.dram_tensor("wqkv", [C, 3 * C], BF16, kind="ExternalInput")
    wproj_t = nc.dram_tensor("wproj", [8, 128, C], BF16, kind="ExternalInput")
    mask_t = nc.dram_tensor("mask", [128, 512], BF16, kind="ExternalInput")
    identb_t = nc.dram_tensor("identb", [128, 128], BF16, kind="ExternalInput")
    
    out_ap = out_t.ap()
    xt_ap = xt_t.ap()
    n_st = T // ST
    n_tt = ST // 128
    n_grp = ST // 8

    with tile.TileContext(nc) as tc, ExitStack() as ctx:
        const = ctx.enter_context(tc.tile_pool(name="const", bufs=1))
        xtp = ctx.enter_context(tc.tile_pool(name="xtp", bufs=2))
        stg = ctx.enter_context(tc.tile_pool(name="stg", bufs=4))
        flats = ctx.enter_context(tc.tile_pool(name="flats", bufs=2))
        attp = ctx.enter_context(tc.tile_pool(name="attp", bufs=3))
        otp = ctx.enter_context(tc.tile_pool(name="otp", bufs=2))
        vp1 = ctx.enter_context(tc.tile_pool(name="vp1", bufs=1))
        ps_att = ctx.enter_context(tc.tile_pool(name="ps_att", bufs=3, space="PSUM"))
        ps_pack = ctx.enter_context(tc.tile_pool(name="ps_pack", bufs=1, space="PSUM"))
        ps_big = ctx.enter_context(tc.tile_pool(name="ps_big", bufs=3, space="PSUM"))
        ps_vp = ctx.enter_context(tc.tile_pool(name="ps_vp", bufs=1, space="PSUM"))

        wq_sb = const.tile([128, 8, 3072], BF16)
        for kk in range(8):
            nc.sync.dma_start(out=wq_sb[:, kk, :],
                              in_=wqkv_t.ap()[kk * 128:(kk + 1) * 128, :])
        wp_sb = const.tile([128, 8, 1024], BF16)
        nc.sync.dma_start(out=wp_sb[:], in_=wproj_t.ap().rearrange("hp c j -> c hp j"))
        mask_sb = const.tile([128, 4, 128], BF16)
        nc.sync.dma_start(out=mask_sb[:], in_=mask_t.ap().rearrange("p (i c) -> p i c", i=4))
        idb = const.tile([128, 128], BF16)
        nc.sync.dma_start(out=idb[:], in_=identb_t.ap())

        for st in range(n_st):
            t0 = st * ST
            # ---- X^T: direct load of host-transposed bf16 x ----
            xt_sb = xtp.tile([128, 8, ST], BF16, tag="xt")
            nc.sync.dma_start(
                out=xt_sb[:],
                in_=xt_ap.rearrange("(kk c) t -> c kk t", c=128)[:, :, t0:t0 + ST])

            # ---- qkv matmuls -> staging -> repack to flats ----
            qf = flats.tile([64, 16, ST], BF16, tag="qf")
            kf = flats.tile([64, 16, ST], BF16, tag="kf")
            # vstage: group-contiguous [p=(g%2)*64+d, tgrp, (gp*8+t)]
            vstage = vp1.tile([128, ST // 8, 64], BF16, tag="vs")
            for jj in range(24):
                qkv_ps = ps_big.tile([128, ST], F32, tag="big")
                for kk in range(8):
                    nc.tensor.matmul(
                        qkv_ps[:], wq_sb[:, kk, jj * 128:(jj + 1) * 128],
                        xt_sb[:, kk, :], start=(kk == 0), stop=(kk == 7))
                if jj < 16:
                    dst = qf if jj < 8 else kf
                    hh = 2 * (jj % 8)
                    sg = stg.tile([128, ST], BF16, tag="stg")
                    nc.vector.tensor_copy(sg[:], qkv_ps[:])
                    eng = nc.sync if jj < 8 else nc.gpsimd
                    eng.dma_start(out=dst[:, hh, :], in_=sg[0:64, :])
                    eng.dma_start(out=dst[:, hh + 1, :], in_=sg[64:128, :])
                else:
                    gp = jj - 16
                    nc.vector.tensor_copy(
                        vstage[:, :, gp * 8:(gp + 1) * 8],
                        qkv_ps[:].rearrange("p (tg t) -> p tg t", t=8))

            # ---- attention groups, batched 4 at a time ----
            kap4_all = kf[:].rearrange(
                "d (gp g1) (gg i t) -> d gg i g1 gp t", g1=2, i=4, t=8)
            qap_all = qf[:].rearrange("d (hp h1) t -> d h1 hp t", h1=2)
            otflats = []
            otdsts = []
            for tt in range(n_tt):
                otf_tile = otp.tile([128, 8, 128], BF16, tag=f"otf{tt}")
                otflats.append(otf_tile)
                otdsts.append(otf_tile[:].rearrange("p hp (gg i t) -> p gg i hp t", i=4, t=8))

            # V-pack: dense runs of PE transposes per half-supertile,
            # one evict + ones column each.
            vpk = vp1.tile([128, n_grp, 65], BF16, tag="vpk")
            hg = n_grp // 4
            for vh in range(4):
                vp_ps = ps_vp.tile([128, hg, 64], BF16, tag="vp")
                for g in range(hg):
                    grp = vh * hg + g
                    nc.tensor.transpose(vp_ps[0:64, g, :], vstage[0:64, grp, :], idb[0:64, 0:64])
                    nc.tensor.transpose(vp_ps[64:128, g, :], vstage[64:128, grp, :], idb[64:128, 64:128])
                nc.scalar.copy(vpk[:, vh * hg:(vh + 1) * hg, 0:64], vp_ps[:])
            nc.vector.memset(vpk[:, :, 64:65], 1.0)

            for g4 in range(n_grp // 4):
                # K-pack for 4 groups in one copy (stationary needs 1 free dim)
                kp4 = attp.tile([64, 4, 128], BF16, tag="kp")
                nc.vector.tensor_copy(
                    kp4[:].rearrange("d i (g1 gp t) -> d i g1 gp t", g1=2, gp=8),
                    kap4_all[:, g4])

                st_ps = ps_att.tile([128, 4, 128], F32, tag="att")
                for i in range(4):
                    tsl = bass.ts(g4 * 4 + i, 8)
                    nc.tensor.matmul(
                        st_ps[:, i, :], kp4[:, i, :], qap_all[:, :, :, tsl],
                        start=True, stop=True)
                e_sb = attp.tile([128, 4, 128], BF16, tag="e")
                nc.scalar.activation(e_sb[:], st_ps[:], AFT.Exp)
                p_bdt = attp.tile([128, 4, 128], BF16, tag="p")
                nc.vector.tensor_mul(p_bdt[:], e_sb[:], mask_sb[:])

                o_ps = ps_att.tile([128, 4, 65], F32, tag="att")
                for i in range(4):
                    nc.tensor.matmul(o_ps[:, i, :], p_bdt[:, i, :],
                                     vpk[:, g4 * 4 + i, :],
                                     start=True, stop=True)
                recip = attp.tile([128, 4, 1], F32, tag="recip")
                nc.vector.reciprocal(recip[:], o_ps[:, :, 64:65])
                o_norm = attp.tile([128, 4, 64], BF16, tag="onorm")
                nc.vector.tensor_mul(
                    o_norm[:], o_ps[:, :, 0:64],
                    _bcast(recip[:, :, 0], 64))

                ot_ps = ps_pack.tile([128, 4, 64], BF16, tag="pack")
                for i in range(4):
                    nc.tensor.transpose(ot_ps[0:64, i, :], o_norm[0:64, i, :], idb[0:64, 0:64])
                    nc.tensor.transpose(ot_ps[64:128, i, :], o_norm[64:128, i, :], idb[64:128, 64:128])
                tt_i, g4_i = divmod(g4, 4)
                nc.scalar.copy(otdsts[tt_i][0:64, g4_i], ot_ps[0:64, :, :])
                nc.scalar.copy(otdsts[tt_i][64:128, g4_i], ot_ps[64:128, :, :])

            # ---- proj ----
            for tt in range(n_tt):
                for nh in range(2):
                    prj_ps = ps_big.tile([128, 512], F32, tag="big")
                    for hp in range(8):
                        nc.tensor.matmul(
                            prj_ps[:], otflats[tt][:, hp, :],
                            wp_sb[:, hp, nh * 512:(nh + 1) * 512],
                            start=(hp == 0), stop=(hp == 7))
                    o_out = stg.tile([128, 512], F32, tag="oout")
                    nc.any.tensor_copy(o_out[:], prj_ps[:])
                    nc.sync.dma_start(
                        out=out_ap[t0 + tt * 128: t0 + (tt + 1) * 128,
                                   nh * 512:(nh + 1) * 512],
                        in_=o_out[:])
    return nc


_CACHE = {}


def _get_nc():
    if "nc" not in _CACHE:
        nc = bacc.Bacc("TRN2", target_bir_lowering=False, debug=False,
                       enable_asserts=False, num_devices=8)
        _build(nc)
        nc.compile()
        _CACHE["nc"] = nc
    return _CACHE["nc"]


def _host_prep(w_qkv, w_proj):
    wqkv_mod = np.asarray(w_qkv, np.float32).copy()
    wqkv_mod[:, :C] *= SCALE
    wqkv_b = wqkv_mod.astype(bf16)
    wproj_b = np.ascontiguousarray(
        np.asarray(w_proj, np.float32).reshape(8, 128, C)).astype(bf16)
    mask = np.zeros((128, 128), np.float32)
    for g in range(H):
        for t in range(8):
            pg = (g % 2) * 64 + (g // 2) * 8 + t
            for h in range(H):
                ph = (h % 2) * 64 + (h // 2) * 8 + t
                mask[pg, ph] = 1.0
    mask4 = np.tile(mask, (1, 4)).astype(bf16)
    identb = np.eye(128, dtype=np.float32).astype(bf16)
    identf = np.eye(128, dtype=np.float32)
    return wqkv_b, wproj_b, mask4, identb, identf


def _run(x, w_qkv, w_proj, b_proj, trace=False, **kw):
    x = np.asarray(x, np.float32)
    wqkv_b, wproj_b, mask4, identb, identf = _host_prep(w_qkv, w_proj)
    nc = _get_nc()
    in_maps = []
    for b in range(B):
        in_maps.append({
            "xt": np.ascontiguousarray(x[b].T).astype(bf16),
            "wqkv": wqkv_b, "wproj": wproj_b, "mask": mask4,
            "identb": identb,
        })
    res = run_bass_kernel_spmd(nc, in_maps, core_ids=list(range(B)),
                               trace=trace, **kw)
    out = np.stack([res.results[b]["out"] for b in range(B)], axis=0)
    out = out + np.asarray(b_proj, np.float32)[None, None, :]
    return out.astype(np.float32), res


def kernel(x, w_qkv, w_proj, b_proj):
    out, _ = _run(x, w_qkv, w_proj, b_proj, trace=False)
    return out
